# revision 26
# baseline (speedup 1.0000x reference)
"""GAT (3-layer, heads=1) fully fused on 8 Trainium2 NeuronCores.

One bass/Tile program per call does everything on device:
  hT = (x @ W1 + b1)^T                       (TensorE, per 128-node tile)
  for each layer l:
    per tile: [ht|es|ed] = h @ [Wg|Wg@a_s|Wg@a_d]   (one matmul per tile)
    write [ht|es] rows into this core's table shard  -> AllGather -> full table
    per dst block: indirect-DMA gather of source rows, exp(leakyrelu(es+ed)),
    pad-masked softmax denom, weighted accumulate, h += out/denom + bg
  y = sigmoid(h @ W2 + b2), per-core partial sum -> [2,1] output per core.

Host: graph preprocessing only (degree-sorted node relabel, 128-node dst
blocks padded to block max degree, snake-dealt to cores; gather indices
point into the AllGather row layout core*6272 + pos*128 + lane). All inputs
are pushed to device once and cached; steady-state calls re-run the full
on-device computation with no per-call host->device traffic (the pre-zeroed
output operand is a persistent device buffer; o is fully DMA-written). The
per-core [2,1] partials are AllReduce'd on device so the host fetches a
single 8-byte shard. Dummy padded nodes (no edges) evolve as h=b1+sum(bg)
exactly; their sigmoid contribution is subtracted on host.
"""

import time

import numpy as np

launch_ns = 0  # cumulative wall time spent in device launches

N = 50000
E = 600000
DH = 128
L = 3
NEG = 0.2
CORES = 8
NB = 392                  # dst blocks of 128 (50176 node slots)
BPC = NB // CORES         # 49 blocks (tiles) per core
SHARD = BPC * 128         # 6272 nodes per core
NPAD = NB * 128           # 50176
SENT = NPAD               # sentinel table row (zeros, one-flag 0)
NTAB = NPAD + 128         # table rows (sentinel block)
DROW = 132                # ht(128) | es | one | pad pad
CHUNK = 16

_cache = {}


# ----------------------------------------------------------------- host prep
def _snake():
    bidx = np.arange(NB)
    posb = bidx // CORES
    kb = bidx % CORES
    coreb = np.where(posb % 2 == 0, kb, CORES - 1 - kb)
    block_cp = np.empty((CORES, BPC), np.int64)
    block_cp[coreb, posb] = bidx
    return coreb, posb, block_cp


def _preprocess(edge_index):
    src = np.asarray(edge_index[0], np.int64)
    dst = np.asarray(edge_index[1], np.int64)
    loops = np.arange(N, dtype=np.int64)
    src = np.concatenate([src, loops])
    dst = np.concatenate([dst, loops])

    deg = np.bincount(dst, minlength=N)
    order = np.argsort(-deg, kind="stable")          # rank -> old id
    rank_of = np.empty(N, np.int64)
    rank_of[order] = np.arange(N)
    rsrc = rank_of[src]
    rdst = rank_of[dst]
    deg_r = deg[order]

    esort = np.argsort(rdst, kind="stable")
    rsrc_s = rsrc[esort]
    starts = np.zeros(NPAD + 1, np.int64)
    starts[1:N + 1] = np.cumsum(deg_r)
    starts[N + 1:] = starts[N]

    degp = np.zeros(NPAD, np.int64)
    degp[:N] = deg_r
    Db = np.maximum(degp.reshape(NB, 128).max(axis=1), 1)

    coreb, posb, block_cp = _snake()
    rounds_per_pos = Db.reshape(BPC, CORES).max(axis=1)
    colpos = np.concatenate([[0], np.cumsum(rounds_per_pos)]).astype(np.int64)
    R = int(colpos[-1])

    # device row of each rank (AllGather layout)
    allrank = np.arange(NPAD)
    b_of_rank = allrank // 128
    row_of_rank = coreb[b_of_rank] * SHARD + posb[b_of_rank] * 128 + (allrank % 128)
    row_of_src = np.concatenate([row_of_rank[:N], [SENT]])  # rank N.. unused

    idx_all = np.full((CORES, 128, R), SENT, np.int32)
    nmax = len(rsrc_s) - 1
    for c in range(CORES):
        for p in range(BPC):
            b = int(block_cp[c, p])
            Dbb = int(Db[b])
            ranks = np.arange(b * 128, (b + 1) * 128)
            d0 = starts[ranks]
            kk = starts[ranks + 1] - d0
            ar = np.arange(Dbb)
            cols = np.minimum(d0[:, None] + ar[None, :], nmax)
            vals = np.where(ar[None, :] < kk[:, None],
                            row_of_rank[rsrc_s[cols]], SENT)
            idx_all[c, :, colpos[p]:colpos[p] + Dbb] = vals

    # old node id feeding each device row (or -1 for dummy)
    rank_of_row = np.empty(NPAD, np.int64)
    rank_of_row[row_of_rank] = allrank
    old_of_row = np.where(rank_of_row < N, order[np.minimum(rank_of_row, N - 1)], -1)

    return {
        "rounds": tuple(int(v) for v in rounds_per_pos),
        "idx_all": idx_all,
        "old_of_row": old_of_row,
        "R": R,
    }


# ------------------------------------------------------------- bass program
def _build_program(rounds_per_pos):
    import concourse.bacc as bacc
    import concourse.mybir as mybir
    import concourse.tile as tile
    from concourse.bass import IndirectOffsetOnAxis, ts
    from concourse.masks import make_identity

    f32 = mybir.dt.float32
    bf16 = mybir.dt.bfloat16
    R = int(sum(rounds_per_pos))
    nc = bacc.Bacc(trn_type="TRN2", num_devices=CORES)

    xin = nc.dram_tensor("xin", [SHARD, DH], f32, kind="ExternalInput")
    idx = nc.dram_tensor("idx", [128, R], mybir.dt.int32, kind="ExternalInput")
    w1 = nc.dram_tensor("w1", [DH, DH], f32, kind="ExternalInput")
    wcat = nc.dram_tensor("wcat", [L * DH, DH + 2], f32, kind="ExternalInput")
    w2 = nc.dram_tensor("w2", [DH, 2], f32, kind="ExternalInput")
    bvec = nc.dram_tensor("bvec", [DH, 1 + L], f32, kind="ExternalInput")
    b2v = nc.dram_tensor("b2v", [2, 1], f32, kind="ExternalInput")
    o = nc.dram_tensor("o", [2, 1], f32, kind="ExternalOutput")

    with tile.TileContext(nc) as tc:
        with (
            tc.tile_pool(name="persist", bufs=1) as pp,
            tc.tile_pool(name="dram", bufs=1, space="DRAM") as dp,
            tc.tile_pool(name="ps", bufs=2, space="PSUM") as ps,
            tc.tile_pool(name="pst", bufs=2, space="PSUM") as pst,
            tc.tile_pool(name="ld", bufs=3) as ld,
            tc.tile_pool(name="g", bufs=6) as gp,
            tc.tile_pool(name="sm", bufs=6) as sm,
            tc.tile_pool(name="blk", bufs=2) as bp,
        ):
            # bf16 table halves AllGather + gather-DMA bytes; es/ht precision
            # loss (~4e-3 relative) is far inside the 2e-2 gate
            shard = dp.tile([SHARD, DROW], bf16, name="shard")
            table = dp.tile([NTAB, DROW], bf16, name="table")

            hT = pp.tile([128, SHARD], f32, name="hT")
            idx_sb = pp.tile([128, R], mybir.dt.int32, name="idx_sb")
            nc.sync.dma_start(idx_sb[:], idx[:, :])
            ident = pp.tile([128, 128], f32, name="ident")
            make_identity(nc, ident[:])
            w1s = pp.tile([128, DH], f32, name="w1s")
            nc.sync.dma_start(w1s[:], w1[:, :])
            wcs = pp.tile([128, L * (DH + 2)], f32, name="wcs")
            nc.sync.dma_start(
                wcs[:].rearrange("p (l d) -> p l d", l=L),
                wcat[:, :].rearrange("(l p) d -> p l d", l=L))
            w2s = pp.tile([128, 2], f32, name="w2s")
            nc.sync.dma_start(w2s[:], w2[:, :])
            bvs = pp.tile([128, 1 + L], f32, name="bvs")
            nc.sync.dma_start(bvs[:], bvec[:, :])
            b2s = pp.tile([2, 1], f32, name="b2s")
            nc.sync.dma_start(b2s[:], b2v[:, :])
            edlane = pp.tile([128, BPC], f32, name="edlane")
            fparts = pp.tile([2, BPC], f32, name="fparts")

            # sentinel block rows + constant one-flag column of the shard
            zrow = pp.tile([128, DROW], bf16, name="zrow")
            nc.vector.memset(zrow[:], 0.0)
            nc.sync.dma_start(table[NPAD:NTAB, :], zrow[:])
            onec = pp.tile([128, BPC], bf16, name="onec")
            nc.vector.memset(onec[:], 1.0)
            nc.sync.dma_start(shard[:, DH + 1:DH + 2], onec[:])

            # ---- mlp1: hT = (x @ W1 + b1)^T
            for t in range(BPC):
                xt = ld.tile([128, DH], f32, tag="xt")
                nc.sync.dma_start(xt[:], xin[ts(t, 128), :])
                xT_ps = pst.tile([128, 128], f32, tag="tr")
                nc.tensor.transpose(xT_ps[:], xt[:], ident[:])
                xT_sb = ld.tile([128, 128], f32, tag="xT_sb")
                nc.vector.tensor_copy(xT_sb[:], xT_ps[:])
                h_ps = ps.tile([128, DH + 2], f32, tag="mm")
                nc.tensor.matmul(out=h_ps[:, 0:128], lhsT=w1s[:], rhs=xT_sb[:],
                                 start=True, stop=True)
                nc.vector.tensor_scalar(
                    out=hT[:, ts(t, 128)], in0=h_ps[:, 0:128], scalar1=bvs[:, 0:1],
                    scalar2=None, op0=mybir.AluOpType.add)

            for l in range(L):
                wc_l = wcs[:].rearrange("p (l d) -> p l d", l=L)[:, l, :]
                # ---- shard build: [ht|es|ed] per tile
                for t in range(BPC):
                    t_ps = ps.tile([128, DH + 2], f32, tag="mm")
                    nc.tensor.matmul(out=t_ps[:], lhsT=hT[:, ts(t, 128)],
                                     rhs=wc_l, start=True, stop=True)
                    row_sb = ld.tile([128, DH + 1], bf16, tag="row_sb")
                    nc.vector.tensor_copy(row_sb[:], t_ps[:, 0:DH + 1])
                    nc.sync.dma_start(shard[ts(t, 128), 0:DH + 1], row_sb[:])
                    nc.vector.tensor_copy(edlane[:, t:t + 1],
                                          t_ps[:, DH + 1:DH + 2])

                nc.gpsimd.collective_compute(
                    "AllGather", mybir.AluOpType.bypass,
                    replica_groups=[list(range(CORES))],
                    ins=[shard[:, :].opt()],
                    outs=[table[0:NPAD, :].opt()],
                )

                # ---- gather + softmax + aggregate per dst block
                col = 0
                for p in range(BPC):
                    Dbp = int(rounds_per_pos[p])
                    nch = (Dbp + CHUNK - 1) // CHUNK
                    Ut = bp.tile([128, DH], f32, tag="Ut")
                    parts = bp.tile([128, CHUNK], f32, tag="parts")
                    nc.vector.memset(Ut[:], 0.0)
                    for ci, c0 in enumerate(range(0, Dbp, CHUNK)):
                        c = min(CHUNK, Dbp - c0)
                        G = gp.tile([128, CHUNK * DROW], bf16, tag="G")
                        # one indirect DMA gathers all c rows per partition:
                        # offset (p, j) -> G[p, j*DROW:(j+1)*DROW]
                        nc.gpsimd.indirect_dma_start(
                            out=G[:, 0:c * DROW],
                            out_offset=None,
                            in_=table[:, :],
                            in_offset=IndirectOffsetOnAxis(
                                ap=idx_sb[:, col + c0:col + c0 + c],
                                axis=0),
                        )
                        G3 = G[:].rearrange("p (c d) -> p c d", d=DROW)
                        z = sm.tile([128, CHUNK], f32, tag="z")
                        nc.vector.tensor_scalar(
                            out=z[:, :c], in0=G3[:, :c, DH],
                            scalar1=edlane[:, p:p + 1], scalar2=None,
                            op0=mybir.AluOpType.add)
                        lr = sm.tile([128, CHUNK], f32, tag="lr")
                        nc.vector.scalar_tensor_tensor(
                            out=lr[:, :c], in0=z[:, :c], scalar=NEG,
                            in1=z[:, :c], op0=mybir.AluOpType.mult,
                            op1=mybir.AluOpType.max)
                        ez = sm.tile([128, CHUNK], f32, tag="ez")
                        nc.scalar.activation(
                            out=ez[:, :c], in_=lr[:, :c],
                            func=mybir.ActivationFunctionType.Exp)
                        exm = sm.tile([128, CHUNK], f32, tag="exm")
                        nc.vector.scalar_tensor_tensor(
                            out=exm[:, :c], in0=ez[:, :c], scalar=1.0,
                            in1=G3[:, :c, DH + 1], op0=mybir.AluOpType.mult,
                            op1=mybir.AluOpType.mult,
                            accum_out=parts[:, ci:ci + 1])
                        for j in range(c):
                            nc.vector.scalar_tensor_tensor(
                                out=Ut[:], in0=G[:, j * DROW:j * DROW + DH],
                                scalar=exm[:, j:j + 1], in1=Ut[:],
                                op0=mybir.AluOpType.mult,
                                op1=mybir.AluOpType.add)
                    dn = bp.tile([128, 1], f32, tag="dn")
                    nc.vector.tensor_reduce(
                        out=dn[:], in_=parts[:, :nch], axis=mybir.AxisListType.X,
                        op=mybir.AluOpType.add)
                    nc.vector.tensor_scalar(
                        out=dn[:], in0=dn[:], scalar1=1e-30, scalar2=None,
                        op0=mybir.AluOpType.max)
                    rc = bp.tile([128, 1], f32, tag="rc")
                    nc.vector.reciprocal(out=rc[:], in_=dn[:])
                    us = bp.tile([128, DH], f32, tag="us")
                    nc.vector.tensor_scalar(
                        out=us[:], in0=Ut[:], scalar1=rc[:, 0:1], scalar2=None,
                        op0=mybir.AluOpType.mult)
                    uT_ps = pst.tile([128, 128], f32, tag="tr")
                    nc.tensor.transpose(uT_ps[:], us[:], ident[:])
                    nc.vector.scalar_tensor_tensor(
                        out=hT[:, ts(p, 128)], in0=uT_ps[:],
                        scalar=bvs[:, 1 + l:2 + l], in1=hT[:, ts(p, 128)],
                        op0=mybir.AluOpType.add, op1=mybir.AluOpType.add)
                    col += Dbp

            # ---- mlp2 + sigmoid + masked partial sum
            for t in range(BPC):
                y_ps = ps.tile([128, DH + 2], f32, tag="mm")
                nc.tensor.matmul(out=y_ps[0:2, 0:128], lhsT=w2s[:],
                                 rhs=hT[:, ts(t, 128)], start=True, stop=True)
                y_sb = sm.tile([2, 128], f32, tag="y_sb")
                nc.scalar.activation(
                    out=y_sb[:], in_=y_ps[0:2, 0:128],
                    func=mybir.ActivationFunctionType.Sigmoid,
                    bias=b2s[:, 0:1], scale=1.0,
                    accum_out=fparts[:, t:t + 1])
            acc = pp.tile([2, 1], f32, name="acc")
            nc.vector.tensor_reduce(
                out=acc[:], in_=fparts[:, :BPC], axis=mybir.AxisListType.X,
                op=mybir.AluOpType.add)
            ob_in = dp.tile([2, 1], f32, name="ob_in")
            ob_out = dp.tile([2, 1], f32, name="ob_out")
            nc.sync.dma_start(ob_in[:, :], acc[:])
            nc.gpsimd.collective_compute(
                "AllReduce", mybir.AluOpType.add,
                replica_groups=[list(range(CORES))],
                ins=[ob_in[:, :].opt()],
                outs=[ob_out[:, :].opt()],
            )
            nc.sync.dma_start(o[:, :], ob_out[:, :])
    nc.finalize()
    return nc


# ------------------------------------------------------------------- runner
def _make_runner(nc):
    import jax
    from jax.experimental.shard_map import shard_map
    from jax.sharding import Mesh, NamedSharding, PartitionSpec
    import concourse.mybir as mybir
    from concourse import bass2jax

    bass2jax.install_neuronx_cc_hook()
    pname = nc.partition_id_tensor.name if nc.partition_id_tensor else None
    in_names, out_names, out_avals, out_shapes = [], [], [], []
    for alloc in nc.m.functions[0].allocations:
        if not isinstance(alloc, mybir.MemoryLocationSet):
            continue
        name = alloc.memorylocations[0].name
        if alloc.kind == "ExternalInput":
            if name != pname:
                in_names.append(name)
        elif alloc.kind == "ExternalOutput":
            out_names.append(name)
            shape = tuple(alloc.tensor_shape)
            dtype = mybir.dt.np(alloc.dtype)
            out_avals.append(jax.core.ShapedArray(shape, dtype))
            out_shapes.append((shape, dtype))
    all_in = in_names + out_names + ([pname] if pname else [])

    def _body(*args):
        operands = list(args)
        if pname:
            operands.append(bass2jax.partition_id_tensor())
        outs = bass2jax._bass_exec_p.bind(
            *operands, out_avals=tuple(out_avals), in_names=tuple(all_in),
            out_names=tuple(out_names), lowering_input_output_aliases=(),
            sim_require_finite=True, sim_require_nnan=True, nc=nc)
        return tuple(outs)

    devices = jax.devices()[:CORES]
    mesh = Mesh(np.asarray(devices), ("core",))
    specs_in = (PartitionSpec("core"),) * (len(in_names) + len(out_names))
    specs_out = (PartitionSpec("core"),) * len(out_names)
    sharded = jax.jit(
        shard_map(_body, mesh=mesh, in_specs=specs_in, out_specs=specs_out,
                  check_rep=False),
        keep_unused=True)
    sharding = NamedSharding(mesh, PartitionSpec("core"))

    dev_cache = {}   # name -> (host_concat_array, device_array)
    # o is fully DMA-written by the program, so the pre-zeroed "output input"
    # can be a single persistent device buffer reused every call (no H2D).
    dev_zeros = jax.device_put(
        [np.zeros((CORES * s[0], *s[1:]), d) for (s, d) in out_shapes],
        [sharding] * len(out_shapes))

    aot = [None]   # AOT-compiled executable, built on first launch

    def run(concat_in: dict | None):
        global launch_ns
        t0 = time.perf_counter()
        if concat_in is not None:
            stale = []
            for nm in in_names:
                ent = dev_cache.get(nm)
                if ent is None or not (
                        ent[0] is concat_in[nm]
                        or np.array_equal(ent[0], concat_in[nm])):
                    stale.append(nm)
            if stale:
                put = jax.device_put([concat_in[nm] for nm in stale],
                                     [sharding] * len(stale))
                for nm, d in zip(stale, put):
                    dev_cache[nm] = (concat_in[nm], d)
        args = [dev_cache[nm][1] for nm in in_names] + list(dev_zeros)
        if aot[0] is None:
            try:
                aot[0] = sharded.lower(*args).compile()
            except Exception:
                aot[0] = sharded   # fall back to the pjit path
        out_arrs = aot[0](*args)
        # o is AllReduce'd on device: every core holds the total; read shard 0
        res = np.asarray(out_arrs[0].addressable_shards[0].data)
        launch_ns += int((time.perf_counter() - t0) * 1e9)
        return res.reshape(2)

    return run


# ------------------------------------------------------------------- kernel
def kernel(x, edge_index, batch, W1, b1, Wg, att_src, att_dst, bg, W2, b2):
    x = np.ascontiguousarray(np.asarray(x, np.float32))
    W1 = np.asarray(W1, np.float32); b1 = np.asarray(b1, np.float32)
    Wg = np.asarray(Wg, np.float32)
    att_src = np.asarray(att_src, np.float32)
    att_dst = np.asarray(att_dst, np.float32)
    bg = np.asarray(bg, np.float32)
    W2 = np.asarray(W2, np.float32); b2 = np.asarray(b2, np.float32)
    ei = np.ascontiguousarray(np.asarray(edge_index))

    # ---- graph preprocessing (cached on edge_index content)
    g = _cache.get("graph")
    if g is None or not (g[0] is ei or np.array_equal(g[0], ei)):
        g = (ei, _preprocess(ei))
        _cache["graph"] = g
    pre = g[1]

    key = pre["rounds"]
    prog = _cache.get("prog")
    if prog is None or prog[0] != key:
        nc = _build_program(pre["rounds"])
        prog = (key, nc, _make_runner(nc))
        _cache["prog"] = prog
    _, nc, run = prog

    # ---- per-call input assembly, memoized on raw input content
    def _same(a, b):
        return a is b or (a.shape == b.shape and np.array_equal(a, b))

    raw = _cache.get("raw")
    wts = (W1, b1, Wg, att_src, att_dst, bg, W2, b2)
    hit = (raw is not None and _same(raw[0], x) and raw[1] is pre
           and all(_same(a, b) for a, b in zip(raw[2], wts)))
    if hit:
        total = run(None)
    else:
        old = pre["old_of_row"]
        xfull = np.zeros((NPAD, DH), np.float32)
        valid = old >= 0
        xfull[valid] = x[old[valid]]

        va_s = np.einsum("lij,lj->li", Wg, att_src)      # [L,DH]
        va_d = np.einsum("lij,lj->li", Wg, att_dst)
        wcat = np.concatenate([Wg, va_s[:, :, None], va_d[:, :, None]],
                              axis=2).reshape(L * DH, DH + 2).astype(np.float32)
        bvec = np.concatenate([b1[:, None], bg.T], axis=1).astype(np.float32)
        b2v = b2.reshape(2, 1).astype(np.float32)

        rep = lambda a: np.concatenate([a] * CORES, axis=0)
        concat_in = {
            "xin": xfull,
            "idx": np.ascontiguousarray(
                pre["idx_all"].reshape(CORES * 128, pre["R"])),
            "w1": rep(W1), "wcat": rep(wcat), "w2": rep(W2),
            "bvec": rep(bvec), "b2v": rep(b2v),
        }
        total = run(concat_in)
        _cache["raw"] = (x, pre, tuple(np.copy(w) for w in wts))

    # exact dummy-node correction: h_dummy = b1 + sum(bg), no edges ever
    h_dummy = b1 + bg.sum(axis=0)
    y_dummy = 1.0 / (1.0 + np.exp(-(h_dummy @ W2 + b2)))
    return (total - (NPAD - N) * y_dummy).astype(np.float32)


# revision 27
# speedup vs baseline: 1.0658x; 1.0658x over previous
"""GAT (3-layer, heads=1) fully fused on 8 Trainium2 NeuronCores.

One bass/Tile program per call does everything on device:
  hT = (x @ W1 + b1)^T                       (TensorE, per 128-node tile)
  for each layer l:
    per tile: [ht|es|ed] = h @ [Wg|Wg@a_s|Wg@a_d]   (one matmul per tile)
    write [ht|es] rows into this core's table shard  -> AllGather -> full table
    per dst block: indirect-DMA gather of source rows, exp(leakyrelu(es+ed)),
    pad-masked softmax denom, weighted accumulate, h += out/denom + bg
  y = sigmoid(h @ W2 + b2), per-core partial sum -> [2,1] output per core.

Host: graph preprocessing only (degree-sorted node relabel, 128-node dst
blocks padded to block max degree, snake-dealt to cores; gather indices
point into the AllGather row layout core*6272 + pos*128 + lane). All inputs
are pushed to device once and cached; steady-state calls re-run the full
on-device computation with no per-call host->device traffic (the pre-zeroed
output operand is a persistent device buffer; o is fully DMA-written). The
per-core [2,1] partials are AllReduce'd on device so the host fetches a
single 8-byte shard. Dummy padded nodes (no edges) evolve as h=b1+sum(bg)
exactly; their sigmoid contribution is subtracted on host.
"""

import time

import numpy as np

launch_ns = 0  # cumulative wall time spent in device launches

N = 50000
E = 600000
DH = 128
L = 3
NEG = 0.2
CORES = 8
NB = 392                  # dst blocks of 128 (50176 node slots)
BPC = NB // CORES         # 49 blocks (tiles) per core
SHARD = BPC * 128         # 6272 nodes per core
NPAD = NB * 128           # 50176
SENT = NPAD               # sentinel table row (zeros, one-flag 0)
NTAB = NPAD + 128         # table rows (sentinel block)
DROW = 132                # ht(128) | es | one | pad pad
CHUNK = 16

_cache = {}


# ----------------------------------------------------------------- host prep
def _snake():
    bidx = np.arange(NB)
    posb = bidx // CORES
    kb = bidx % CORES
    coreb = np.where(posb % 2 == 0, kb, CORES - 1 - kb)
    block_cp = np.empty((CORES, BPC), np.int64)
    block_cp[coreb, posb] = bidx
    return coreb, posb, block_cp


def _preprocess(edge_index):
    src = np.asarray(edge_index[0], np.int64)
    dst = np.asarray(edge_index[1], np.int64)
    loops = np.arange(N, dtype=np.int64)
    src = np.concatenate([src, loops])
    dst = np.concatenate([dst, loops])

    deg = np.bincount(dst, minlength=N)
    order = np.argsort(-deg, kind="stable")          # rank -> old id
    rank_of = np.empty(N, np.int64)
    rank_of[order] = np.arange(N)
    rsrc = rank_of[src]
    rdst = rank_of[dst]
    deg_r = deg[order]

    esort = np.argsort(rdst, kind="stable")
    rsrc_s = rsrc[esort]
    starts = np.zeros(NPAD + 1, np.int64)
    starts[1:N + 1] = np.cumsum(deg_r)
    starts[N + 1:] = starts[N]

    degp = np.zeros(NPAD, np.int64)
    degp[:N] = deg_r
    Db = np.maximum(degp.reshape(NB, 128).max(axis=1), 1)

    coreb, posb, block_cp = _snake()
    rounds_per_pos = Db.reshape(BPC, CORES).max(axis=1)
    colpos = np.concatenate([[0], np.cumsum(rounds_per_pos)]).astype(np.int64)
    R = int(colpos[-1])

    # device row of each rank (AllGather layout)
    allrank = np.arange(NPAD)
    b_of_rank = allrank // 128
    row_of_rank = coreb[b_of_rank] * SHARD + posb[b_of_rank] * 128 + (allrank % 128)
    row_of_src = np.concatenate([row_of_rank[:N], [SENT]])  # rank N.. unused

    idx_all = np.full((CORES, 128, R), SENT, np.int32)
    nmax = len(rsrc_s) - 1
    for c in range(CORES):
        for p in range(BPC):
            b = int(block_cp[c, p])
            Dbb = int(Db[b])
            ranks = np.arange(b * 128, (b + 1) * 128)
            d0 = starts[ranks]
            kk = starts[ranks + 1] - d0
            ar = np.arange(Dbb)
            cols = np.minimum(d0[:, None] + ar[None, :], nmax)
            vals = np.where(ar[None, :] < kk[:, None],
                            row_of_rank[rsrc_s[cols]], SENT)
            idx_all[c, :, colpos[p]:colpos[p] + Dbb] = vals

    # old node id feeding each device row (or -1 for dummy)
    rank_of_row = np.empty(NPAD, np.int64)
    rank_of_row[row_of_rank] = allrank
    old_of_row = np.where(rank_of_row < N, order[np.minimum(rank_of_row, N - 1)], -1)

    return {
        "rounds": tuple(int(v) for v in rounds_per_pos),
        "idx_all": idx_all,
        "old_of_row": old_of_row,
        "R": R,
    }


# ------------------------------------------------------------- bass program
def _build_program(rounds_per_pos):
    import concourse.bacc as bacc
    import concourse.mybir as mybir
    import concourse.tile as tile
    from concourse.bass import IndirectOffsetOnAxis, ts
    from concourse.masks import make_identity

    f32 = mybir.dt.float32
    bf16 = mybir.dt.bfloat16
    R = int(sum(rounds_per_pos))
    nc = bacc.Bacc(trn_type="TRN2", num_devices=CORES)

    xin = nc.dram_tensor("xin", [SHARD, DH], f32, kind="ExternalInput")
    idx = nc.dram_tensor("idx", [128, R], mybir.dt.int32, kind="ExternalInput")
    w1 = nc.dram_tensor("w1", [DH, DH], f32, kind="ExternalInput")
    wcat = nc.dram_tensor("wcat", [L * DH, DH + 2], f32, kind="ExternalInput")
    w2 = nc.dram_tensor("w2", [DH, 2], f32, kind="ExternalInput")
    bvec = nc.dram_tensor("bvec", [DH, 1 + L], f32, kind="ExternalInput")
    b2v = nc.dram_tensor("b2v", [2, 1], f32, kind="ExternalInput")
    o = nc.dram_tensor("o", [2, 1], f32, kind="ExternalOutput")

    with tile.TileContext(nc) as tc:
        with (
            tc.tile_pool(name="persist", bufs=1) as pp,
            tc.tile_pool(name="dram", bufs=1, space="DRAM") as dp,
            tc.tile_pool(name="ps", bufs=2, space="PSUM") as ps,
            tc.tile_pool(name="pst", bufs=2, space="PSUM") as pst,
            tc.tile_pool(name="ld", bufs=3) as ld,
            tc.tile_pool(name="g", bufs=6) as gp,
            tc.tile_pool(name="sm", bufs=6) as sm,
            tc.tile_pool(name="blk", bufs=2) as bp,
        ):
            # bf16 table halves AllGather + gather-DMA bytes; es/ht precision
            # loss (~4e-3 relative) is far inside the 2e-2 gate
            shard = dp.tile([SHARD, DROW], bf16, name="shard")
            table = dp.tile([NTAB, DROW], bf16, name="table")

            hT = pp.tile([128, SHARD], f32, name="hT")
            idx_sb = pp.tile([128, R], mybir.dt.int32, name="idx_sb")
            nc.sync.dma_start(idx_sb[:], idx[:, :])
            ident = pp.tile([128, 128], f32, name="ident")
            make_identity(nc, ident[:])
            w1s = pp.tile([128, DH], f32, name="w1s")
            nc.sync.dma_start(w1s[:], w1[:, :])
            wcs = pp.tile([128, L * (DH + 2)], f32, name="wcs")
            nc.sync.dma_start(
                wcs[:].rearrange("p (l d) -> p l d", l=L),
                wcat[:, :].rearrange("(l p) d -> p l d", l=L))
            w2s = pp.tile([128, 2], f32, name="w2s")
            nc.sync.dma_start(w2s[:], w2[:, :])
            bvs = pp.tile([128, 1 + L], f32, name="bvs")
            nc.sync.dma_start(bvs[:], bvec[:, :])
            b2s = pp.tile([2, 1], f32, name="b2s")
            nc.sync.dma_start(b2s[:], b2v[:, :])
            edlane = pp.tile([128, BPC], f32, name="edlane")
            fparts = pp.tile([2, BPC], f32, name="fparts")

            # sentinel block rows + constant one-flag column of the shard
            zrow = pp.tile([128, DROW], bf16, name="zrow")
            nc.vector.memset(zrow[:], 0.0)
            nc.sync.dma_start(table[NPAD:NTAB, :], zrow[:])
            onec = pp.tile([128, BPC], bf16, name="onec")
            nc.vector.memset(onec[:], 1.0)
            nc.sync.dma_start(shard[:, DH + 1:DH + 2], onec[:])

            # ---- mlp1: hT = (x @ W1 + b1)^T
            for t in range(BPC):
                xt = ld.tile([128, DH], f32, tag="xt")
                nc.sync.dma_start(xt[:], xin[ts(t, 128), :])
                xT_ps = pst.tile([128, 128], f32, tag="tr")
                nc.tensor.transpose(xT_ps[:], xt[:], ident[:])
                xT_sb = ld.tile([128, 128], f32, tag="xT_sb")
                nc.vector.tensor_copy(xT_sb[:], xT_ps[:])
                h_ps = ps.tile([128, DH + 2], f32, tag="mm")
                nc.tensor.matmul(out=h_ps[:, 0:128], lhsT=w1s[:], rhs=xT_sb[:],
                                 start=True, stop=True)
                nc.vector.tensor_scalar(
                    out=hT[:, ts(t, 128)], in0=h_ps[:, 0:128], scalar1=bvs[:, 0:1],
                    scalar2=None, op0=mybir.AluOpType.add)

            for l in range(L):
                wc_l = wcs[:].rearrange("p (l d) -> p l d", l=L)[:, l, :]
                # ---- shard build: [ht|es|ed] per tile
                for t in range(BPC):
                    t_ps = ps.tile([128, DH + 2], f32, tag="mm")
                    nc.tensor.matmul(out=t_ps[:], lhsT=hT[:, ts(t, 128)],
                                     rhs=wc_l, start=True, stop=True)
                    row_sb = ld.tile([128, DH + 1], bf16, tag="row_sb")
                    nc.vector.tensor_copy(row_sb[:], t_ps[:, 0:DH + 1])
                    nc.sync.dma_start(shard[ts(t, 128), 0:DH + 1], row_sb[:])
                    nc.vector.tensor_copy(edlane[:, t:t + 1],
                                          t_ps[:, DH + 1:DH + 2])

                nc.gpsimd.collective_compute(
                    "AllGather", mybir.AluOpType.bypass,
                    replica_groups=[list(range(CORES))],
                    ins=[shard[:, :].opt()],
                    outs=[table[0:NPAD, :].opt()],
                )

                # ---- gather + softmax + aggregate per dst block
                col = 0
                for p in range(BPC):
                    Dbp = int(rounds_per_pos[p])
                    nch = (Dbp + CHUNK - 1) // CHUNK
                    Ut = bp.tile([128, DH], f32, tag="Ut")
                    parts = bp.tile([128, CHUNK], f32, tag="parts")
                    nc.vector.memset(Ut[:], 0.0)
                    for ci, c0 in enumerate(range(0, Dbp, CHUNK)):
                        c = min(CHUNK, Dbp - c0)
                        G = gp.tile([128, CHUNK * DROW], bf16, tag="G")
                        for j in range(c):
                            nc.gpsimd.indirect_dma_start(
                                out=G[:, j * DROW:(j + 1) * DROW],
                                out_offset=None,
                                in_=table[:, :],
                                in_offset=IndirectOffsetOnAxis(
                                    ap=idx_sb[:, col + c0 + j:col + c0 + j + 1],
                                    axis=0),
                            )
                        G3 = G[:].rearrange("p (c d) -> p c d", d=DROW)
                        z = sm.tile([128, CHUNK], f32, tag="z")
                        nc.vector.tensor_scalar(
                            out=z[:, :c], in0=G3[:, :c, DH],
                            scalar1=edlane[:, p:p + 1], scalar2=None,
                            op0=mybir.AluOpType.add)
                        lr = sm.tile([128, CHUNK], f32, tag="lr")
                        nc.vector.scalar_tensor_tensor(
                            out=lr[:, :c], in0=z[:, :c], scalar=NEG,
                            in1=z[:, :c], op0=mybir.AluOpType.mult,
                            op1=mybir.AluOpType.max)
                        ez = sm.tile([128, CHUNK], f32, tag="ez")
                        nc.scalar.activation(
                            out=ez[:, :c], in_=lr[:, :c],
                            func=mybir.ActivationFunctionType.Exp)
                        exm = sm.tile([128, CHUNK], f32, tag="exm")
                        nc.vector.scalar_tensor_tensor(
                            out=exm[:, :c], in0=ez[:, :c], scalar=1.0,
                            in1=G3[:, :c, DH + 1], op0=mybir.AluOpType.mult,
                            op1=mybir.AluOpType.mult,
                            accum_out=parts[:, ci:ci + 1])
                        for j in range(c):
                            nc.vector.scalar_tensor_tensor(
                                out=Ut[:], in0=G[:, j * DROW:j * DROW + DH],
                                scalar=exm[:, j:j + 1], in1=Ut[:],
                                op0=mybir.AluOpType.mult,
                                op1=mybir.AluOpType.add)
                    dn = bp.tile([128, 1], f32, tag="dn")
                    nc.vector.tensor_reduce(
                        out=dn[:], in_=parts[:, :nch], axis=mybir.AxisListType.X,
                        op=mybir.AluOpType.add)
                    nc.vector.tensor_scalar(
                        out=dn[:], in0=dn[:], scalar1=1e-30, scalar2=None,
                        op0=mybir.AluOpType.max)
                    rc = bp.tile([128, 1], f32, tag="rc")
                    nc.vector.reciprocal(out=rc[:], in_=dn[:])
                    us = bp.tile([128, DH], f32, tag="us")
                    nc.vector.tensor_scalar(
                        out=us[:], in0=Ut[:], scalar1=rc[:, 0:1], scalar2=None,
                        op0=mybir.AluOpType.mult)
                    uT_ps = pst.tile([128, 128], f32, tag="tr")
                    nc.tensor.transpose(uT_ps[:], us[:], ident[:])
                    nc.vector.scalar_tensor_tensor(
                        out=hT[:, ts(p, 128)], in0=uT_ps[:],
                        scalar=bvs[:, 1 + l:2 + l], in1=hT[:, ts(p, 128)],
                        op0=mybir.AluOpType.add, op1=mybir.AluOpType.add)
                    col += Dbp

            # ---- mlp2 + sigmoid + masked partial sum
            for t in range(BPC):
                y_ps = ps.tile([128, DH + 2], f32, tag="mm")
                nc.tensor.matmul(out=y_ps[0:2, 0:128], lhsT=w2s[:],
                                 rhs=hT[:, ts(t, 128)], start=True, stop=True)
                y_sb = sm.tile([2, 128], f32, tag="y_sb")
                nc.scalar.activation(
                    out=y_sb[:], in_=y_ps[0:2, 0:128],
                    func=mybir.ActivationFunctionType.Sigmoid,
                    bias=b2s[:, 0:1], scale=1.0,
                    accum_out=fparts[:, t:t + 1])
            acc = pp.tile([2, 1], f32, name="acc")
            nc.vector.tensor_reduce(
                out=acc[:], in_=fparts[:, :BPC], axis=mybir.AxisListType.X,
                op=mybir.AluOpType.add)
            ob_in = dp.tile([2, 1], f32, name="ob_in")
            ob_out = dp.tile([2, 1], f32, name="ob_out")
            nc.sync.dma_start(ob_in[:, :], acc[:])
            nc.gpsimd.collective_compute(
                "AllReduce", mybir.AluOpType.add,
                replica_groups=[list(range(CORES))],
                ins=[ob_in[:, :].opt()],
                outs=[ob_out[:, :].opt()],
            )
            nc.sync.dma_start(o[:, :], ob_out[:, :])
    nc.finalize()
    return nc


# ------------------------------------------------------------------- runner
def _make_runner(nc):
    import jax
    from jax.experimental.shard_map import shard_map
    from jax.sharding import Mesh, NamedSharding, PartitionSpec
    import concourse.mybir as mybir
    from concourse import bass2jax

    bass2jax.install_neuronx_cc_hook()
    pname = nc.partition_id_tensor.name if nc.partition_id_tensor else None
    in_names, out_names, out_avals, out_shapes = [], [], [], []
    for alloc in nc.m.functions[0].allocations:
        if not isinstance(alloc, mybir.MemoryLocationSet):
            continue
        name = alloc.memorylocations[0].name
        if alloc.kind == "ExternalInput":
            if name != pname:
                in_names.append(name)
        elif alloc.kind == "ExternalOutput":
            out_names.append(name)
            shape = tuple(alloc.tensor_shape)
            dtype = mybir.dt.np(alloc.dtype)
            out_avals.append(jax.core.ShapedArray(shape, dtype))
            out_shapes.append((shape, dtype))
    all_in = in_names + out_names + ([pname] if pname else [])

    def _body(*args):
        operands = list(args)
        if pname:
            operands.append(bass2jax.partition_id_tensor())
        outs = bass2jax._bass_exec_p.bind(
            *operands, out_avals=tuple(out_avals), in_names=tuple(all_in),
            out_names=tuple(out_names), lowering_input_output_aliases=(),
            sim_require_finite=True, sim_require_nnan=True, nc=nc)
        return tuple(outs)

    devices = jax.devices()[:CORES]
    mesh = Mesh(np.asarray(devices), ("core",))
    specs_in = (PartitionSpec("core"),) * (len(in_names) + len(out_names))
    specs_out = (PartitionSpec("core"),) * len(out_names)
    sharded = jax.jit(
        shard_map(_body, mesh=mesh, in_specs=specs_in, out_specs=specs_out,
                  check_rep=False),
        keep_unused=True)
    sharding = NamedSharding(mesh, PartitionSpec("core"))

    dev_cache = {}   # name -> (host_concat_array, device_array)
    # o is fully DMA-written by the program, so the pre-zeroed "output input"
    # can be a single persistent device buffer reused every call (no H2D).
    dev_zeros = jax.device_put(
        [np.zeros((CORES * s[0], *s[1:]), d) for (s, d) in out_shapes],
        [sharding] * len(out_shapes))

    aot = [None]   # AOT-compiled executable, built on first launch

    def run(concat_in: dict | None):
        global launch_ns
        t0 = time.perf_counter()
        if concat_in is not None:
            stale = []
            for nm in in_names:
                ent = dev_cache.get(nm)
                if ent is None or not (
                        ent[0] is concat_in[nm]
                        or np.array_equal(ent[0], concat_in[nm])):
                    stale.append(nm)
            if stale:
                put = jax.device_put([concat_in[nm] for nm in stale],
                                     [sharding] * len(stale))
                for nm, d in zip(stale, put):
                    dev_cache[nm] = (concat_in[nm], d)
        args = [dev_cache[nm][1] for nm in in_names] + list(dev_zeros)
        if aot[0] is None:
            try:
                aot[0] = sharded.lower(*args).compile()
            except Exception:
                aot[0] = sharded   # fall back to the pjit path
        out_arrs = aot[0](*args)
        # o is AllReduce'd on device: every core holds the total; read shard 0
        res = np.asarray(out_arrs[0].addressable_shards[0].data)
        launch_ns += int((time.perf_counter() - t0) * 1e9)
        return res.reshape(2)

    return run


# ------------------------------------------------------------------- kernel
def kernel(x, edge_index, batch, W1, b1, Wg, att_src, att_dst, bg, W2, b2):
    x = np.ascontiguousarray(np.asarray(x, np.float32))
    W1 = np.asarray(W1, np.float32); b1 = np.asarray(b1, np.float32)
    Wg = np.asarray(Wg, np.float32)
    att_src = np.asarray(att_src, np.float32)
    att_dst = np.asarray(att_dst, np.float32)
    bg = np.asarray(bg, np.float32)
    W2 = np.asarray(W2, np.float32); b2 = np.asarray(b2, np.float32)
    ei = np.ascontiguousarray(np.asarray(edge_index))

    # ---- graph preprocessing (cached on edge_index content)
    g = _cache.get("graph")
    if g is None or not (g[0] is ei or np.array_equal(g[0], ei)):
        g = (ei, _preprocess(ei))
        _cache["graph"] = g
    pre = g[1]

    key = pre["rounds"]
    prog = _cache.get("prog")
    if prog is None or prog[0] != key:
        nc = _build_program(pre["rounds"])
        prog = (key, nc, _make_runner(nc))
        _cache["prog"] = prog
    _, nc, run = prog

    # ---- per-call input assembly, memoized on raw input content
    def _same(a, b):
        return a is b or (a.shape == b.shape and np.array_equal(a, b))

    raw = _cache.get("raw")
    wts = (W1, b1, Wg, att_src, att_dst, bg, W2, b2)
    hit = (raw is not None and _same(raw[0], x) and raw[1] is pre
           and all(_same(a, b) for a, b in zip(raw[2], wts)))
    if hit:
        total = run(None)
    else:
        old = pre["old_of_row"]
        xfull = np.zeros((NPAD, DH), np.float32)
        valid = old >= 0
        xfull[valid] = x[old[valid]]

        va_s = np.einsum("lij,lj->li", Wg, att_src)      # [L,DH]
        va_d = np.einsum("lij,lj->li", Wg, att_dst)
        wcat = np.concatenate([Wg, va_s[:, :, None], va_d[:, :, None]],
                              axis=2).reshape(L * DH, DH + 2).astype(np.float32)
        bvec = np.concatenate([b1[:, None], bg.T], axis=1).astype(np.float32)
        b2v = b2.reshape(2, 1).astype(np.float32)

        rep = lambda a: np.concatenate([a] * CORES, axis=0)
        concat_in = {
            "xin": xfull,
            "idx": np.ascontiguousarray(
                pre["idx_all"].reshape(CORES * 128, pre["R"])),
            "w1": rep(W1), "wcat": rep(wcat), "w2": rep(W2),
            "bvec": rep(bvec), "b2v": rep(b2v),
        }
        total = run(concat_in)
        _cache["raw"] = (x, pre, tuple(np.copy(w) for w in wts))

    # exact dummy-node correction: h_dummy = b1 + sum(bg), no edges ever
    h_dummy = b1 + bg.sum(axis=0)
    y_dummy = 1.0 / (1.0 + np.exp(-(h_dummy @ W2 + b2)))
    return (total - (NPAD - N) * y_dummy).astype(np.float32)


# revision 29
# speedup vs baseline: 2.2296x; 2.0920x over previous
"""GAT (3-layer, heads=1) fully fused on 8 Trainium2 NeuronCores.

One bass/Tile program per call does everything on device:
  hT = (x @ W1 + b1)^T                       (TensorE, per 128-node tile)
  for each layer l:
    per tile: [ht|es|ed] = h @ [Wg|Wg@a_s|Wg@a_d]   (one matmul per tile)
    write [ht|es] rows into this core's table shard  -> AllGather -> full table
    per dst block: indirect-DMA gather of source rows, exp(leakyrelu(es+ed)),
    pad-masked softmax denom, weighted accumulate, h += out/denom + bg
  y = sigmoid(h @ W2 + b2), per-core partial sum -> [2,1] output per core.

Host: graph preprocessing only (degree-sorted node relabel, 128-node dst
blocks padded to block max degree, snake-dealt to cores; gather indices
point into the AllGather row layout core*6272 + pos*128 + lane). All inputs
are pushed to device once and cached; steady-state calls re-run the full
on-device computation with no per-call host->device traffic (the pre-zeroed
output operand is a persistent device buffer; o is fully DMA-written). The
per-core [2,1] partials are AllReduce'd on device so the host fetches a
single 8-byte shard. Dummy padded nodes (no edges) evolve as h=b1+sum(bg)
exactly; their sigmoid contribution is subtracted on host.
"""

import time

import numpy as np

launch_ns = 0  # cumulative wall time spent in device launches

N = 50000
E = 600000
DH = 128
L = 3
NEG = 0.2
CORES = 8
NB = 392                  # dst blocks of 128 (50176 node slots)
BPC = NB // CORES         # 49 blocks (tiles) per core
SHARD = BPC * 128         # 6272 nodes per core
NPAD = NB * 128           # 50176
SENT = NPAD               # sentinel table row (zeros, one-flag 0)
NTAB = NPAD + 128         # table rows (sentinel block)
DROW = 132                # ht(128) | es | one | pad pad
CHUNK = 32

_cache = {}


# ----------------------------------------------------------------- host prep
def _snake():
    bidx = np.arange(NB)
    posb = bidx // CORES
    kb = bidx % CORES
    coreb = np.where(posb % 2 == 0, kb, CORES - 1 - kb)
    block_cp = np.empty((CORES, BPC), np.int64)
    block_cp[coreb, posb] = bidx
    return coreb, posb, block_cp


def _preprocess(edge_index):
    src = np.asarray(edge_index[0], np.int64)
    dst = np.asarray(edge_index[1], np.int64)
    loops = np.arange(N, dtype=np.int64)
    src = np.concatenate([src, loops])
    dst = np.concatenate([dst, loops])

    deg = np.bincount(dst, minlength=N)
    order = np.argsort(-deg, kind="stable")          # rank -> old id
    rank_of = np.empty(N, np.int64)
    rank_of[order] = np.arange(N)
    rsrc = rank_of[src]
    rdst = rank_of[dst]
    deg_r = deg[order]

    esort = np.argsort(rdst, kind="stable")
    rsrc_s = rsrc[esort]
    starts = np.zeros(NPAD + 1, np.int64)
    starts[1:N + 1] = np.cumsum(deg_r)
    starts[N + 1:] = starts[N]

    degp = np.zeros(NPAD, np.int64)
    degp[:N] = deg_r
    Db = np.maximum(degp.reshape(NB, 128).max(axis=1), 1)

    coreb, posb, block_cp = _snake()
    rounds_per_pos = Db.reshape(BPC, CORES).max(axis=1)
    colpos = np.concatenate([[0], np.cumsum(rounds_per_pos)]).astype(np.int64)
    R = int(colpos[-1])

    # device row of each rank (AllGather layout)
    allrank = np.arange(NPAD)
    b_of_rank = allrank // 128
    row_of_rank = coreb[b_of_rank] * SHARD + posb[b_of_rank] * 128 + (allrank % 128)
    row_of_src = np.concatenate([row_of_rank[:N], [SENT]])  # rank N.. unused

    idx_all = np.full((CORES, 128, R), SENT, np.int32)
    nmax = len(rsrc_s) - 1
    for c in range(CORES):
        for p in range(BPC):
            b = int(block_cp[c, p])
            Dbb = int(Db[b])
            ranks = np.arange(b * 128, (b + 1) * 128)
            d0 = starts[ranks]
            kk = starts[ranks + 1] - d0
            ar = np.arange(Dbb)
            cols = np.minimum(d0[:, None] + ar[None, :], nmax)
            vals = np.where(ar[None, :] < kk[:, None],
                            row_of_rank[rsrc_s[cols]], SENT)
            idx_all[c, :, colpos[p]:colpos[p] + Dbb] = vals

    # old node id feeding each device row (or -1 for dummy)
    rank_of_row = np.empty(NPAD, np.int64)
    rank_of_row[row_of_rank] = allrank
    old_of_row = np.where(rank_of_row < N, order[np.minimum(rank_of_row, N - 1)], -1)

    return {
        "rounds": tuple(int(v) for v in rounds_per_pos),
        "idx_all": idx_all,
        "old_of_row": old_of_row,
        "R": R,
    }


# ------------------------------------------------------------- bass program
def _build_program(rounds_per_pos):
    import concourse.bacc as bacc
    import concourse.mybir as mybir
    import concourse.tile as tile
    from concourse.bass import IndirectOffsetOnAxis, ts
    from concourse.masks import make_identity

    f32 = mybir.dt.float32
    bf16 = mybir.dt.bfloat16
    R = int(sum(rounds_per_pos))
    nc = bacc.Bacc(trn_type="TRN2", num_devices=CORES)

    xin = nc.dram_tensor("xin", [SHARD, DH], f32, kind="ExternalInput")
    idx = nc.dram_tensor("idx", [128, R], mybir.dt.int32, kind="ExternalInput")
    w1 = nc.dram_tensor("w1", [DH, DH], f32, kind="ExternalInput")
    wcat = nc.dram_tensor("wcat", [L * DH, DH + 2], f32, kind="ExternalInput")
    w2 = nc.dram_tensor("w2", [DH, 2], f32, kind="ExternalInput")
    bvec = nc.dram_tensor("bvec", [DH, 1 + L], f32, kind="ExternalInput")
    b2v = nc.dram_tensor("b2v", [2, 1], f32, kind="ExternalInput")
    o = nc.dram_tensor("o", [2, 1], f32, kind="ExternalOutput")

    with tile.TileContext(nc) as tc:
        with (
            tc.tile_pool(name="persist", bufs=1) as pp,
            tc.tile_pool(name="dram", bufs=1, space="DRAM") as dp,
            tc.tile_pool(name="ps", bufs=2, space="PSUM") as ps,
            tc.tile_pool(name="pst", bufs=2, space="PSUM") as pst,
            tc.tile_pool(name="ld", bufs=3) as ld,
            tc.tile_pool(name="g", bufs=8) as gp,
            tc.tile_pool(name="sm", bufs=6) as sm,
            tc.tile_pool(name="blk", bufs=2) as bp,
        ):
            # bf16 table halves AllGather + gather-DMA bytes; es/ht precision
            # loss (~4e-3 relative) is far inside the 2e-2 gate
            shard = dp.tile([SHARD, DROW], bf16, name="shard")
            table = dp.tile([NTAB, DROW], bf16, name="table")

            hT = pp.tile([128, SHARD], f32, name="hT")
            idx_sb = pp.tile([128, R], mybir.dt.int32, name="idx_sb")
            nc.sync.dma_start(idx_sb[:], idx[:, :])
            ident = pp.tile([128, 128], f32, name="ident")
            make_identity(nc, ident[:])
            w1s = pp.tile([128, DH], f32, name="w1s")
            nc.sync.dma_start(w1s[:], w1[:, :])
            wcs = pp.tile([128, L * (DH + 2)], f32, name="wcs")
            nc.sync.dma_start(
                wcs[:].rearrange("p (l d) -> p l d", l=L),
                wcat[:, :].rearrange("(l p) d -> p l d", l=L))
            w2s = pp.tile([128, 2], f32, name="w2s")
            nc.sync.dma_start(w2s[:], w2[:, :])
            bvs = pp.tile([128, 1 + L], f32, name="bvs")
            nc.sync.dma_start(bvs[:], bvec[:, :])
            b2s = pp.tile([2, 1], f32, name="b2s")
            nc.sync.dma_start(b2s[:], b2v[:, :])
            edlane = pp.tile([128, BPC], f32, name="edlane")
            fparts = pp.tile([2, BPC], f32, name="fparts")

            # sentinel block rows + constant one-flag column of the shard
            zrow = pp.tile([128, DROW], bf16, name="zrow")
            nc.vector.memset(zrow[:], 0.0)
            nc.sync.dma_start(table[NPAD:NTAB, :], zrow[:])
            onec = pp.tile([128, BPC], bf16, name="onec")
            nc.vector.memset(onec[:], 1.0)
            nc.sync.dma_start(shard[:, DH + 1:DH + 2], onec[:])

            # ---- mlp1: hT = (x @ W1 + b1)^T
            for t in range(BPC):
                xt = ld.tile([128, DH], f32, tag="xt")
                nc.sync.dma_start(xt[:], xin[ts(t, 128), :])
                xT_ps = pst.tile([128, 128], f32, tag="tr")
                nc.tensor.transpose(xT_ps[:], xt[:], ident[:])
                xT_sb = ld.tile([128, 128], f32, tag="xT_sb")
                nc.vector.tensor_copy(xT_sb[:], xT_ps[:])
                h_ps = ps.tile([128, DH + 2], f32, tag="mm")
                nc.tensor.matmul(out=h_ps[:, 0:128], lhsT=w1s[:], rhs=xT_sb[:],
                                 start=True, stop=True)
                nc.vector.tensor_scalar(
                    out=hT[:, ts(t, 128)], in0=h_ps[:, 0:128], scalar1=bvs[:, 0:1],
                    scalar2=None, op0=mybir.AluOpType.add)

            for l in range(L):
                wc_l = wcs[:].rearrange("p (l d) -> p l d", l=L)[:, l, :]
                # ---- shard build: [ht|es|ed] per tile
                for t in range(BPC):
                    t_ps = ps.tile([128, DH + 2], f32, tag="mm")
                    nc.tensor.matmul(out=t_ps[:], lhsT=hT[:, ts(t, 128)],
                                     rhs=wc_l, start=True, stop=True)
                    row_sb = ld.tile([128, DH + 1], bf16, tag="row_sb")
                    nc.vector.tensor_copy(row_sb[:], t_ps[:, 0:DH + 1])
                    nc.sync.dma_start(shard[ts(t, 128), 0:DH + 1], row_sb[:])
                    nc.vector.tensor_copy(edlane[:, t:t + 1],
                                          t_ps[:, DH + 1:DH + 2])

                nc.gpsimd.collective_compute(
                    "AllGather", mybir.AluOpType.bypass,
                    replica_groups=[list(range(CORES))],
                    ins=[shard[:, :].opt()],
                    outs=[table[0:NPAD, :].opt()],
                )

                # ---- gather + softmax + aggregate per dst block
                col = 0
                for p in range(BPC):
                    Dbp = int(rounds_per_pos[p])
                    nch = (Dbp + CHUNK - 1) // CHUNK
                    Ut = bp.tile([128, DH], f32, tag="Ut")
                    parts = bp.tile([128, CHUNK], f32, tag="parts")
                    nc.vector.memset(Ut[:], 0.0)
                    for ci, c0 in enumerate(range(0, Dbp, CHUNK)):
                        c = min(CHUNK, Dbp - c0)
                        G = gp.tile([128, CHUNK * DROW], bf16, tag="G")
                        for j in range(c):
                            nc.gpsimd.indirect_dma_start(
                                out=G[:, j * DROW:(j + 1) * DROW],
                                out_offset=None,
                                in_=table[:, :],
                                in_offset=IndirectOffsetOnAxis(
                                    ap=idx_sb[:, col + c0 + j:col + c0 + j + 1],
                                    axis=0),
                            )
                        G3 = G[:].rearrange("p (c d) -> p c d", d=DROW)
                        z = sm.tile([128, CHUNK], f32, tag="z")
                        nc.vector.tensor_scalar(
                            out=z[:, :c], in0=G3[:, :c, DH],
                            scalar1=edlane[:, p:p + 1], scalar2=None,
                            op0=mybir.AluOpType.add)
                        lr = sm.tile([128, CHUNK], f32, tag="lr")
                        nc.vector.scalar_tensor_tensor(
                            out=lr[:, :c], in0=z[:, :c], scalar=NEG,
                            in1=z[:, :c], op0=mybir.AluOpType.mult,
                            op1=mybir.AluOpType.max)
                        ez = sm.tile([128, CHUNK], f32, tag="ez")
                        nc.scalar.activation(
                            out=ez[:, :c], in_=lr[:, :c],
                            func=mybir.ActivationFunctionType.Exp)
                        exm = sm.tile([128, CHUNK], f32, tag="exm")
                        nc.vector.scalar_tensor_tensor(
                            out=exm[:, :c], in0=ez[:, :c], scalar=1.0,
                            in1=G3[:, :c, DH + 1], op0=mybir.AluOpType.mult,
                            op1=mybir.AluOpType.mult,
                            accum_out=parts[:, ci:ci + 1])
                        for j in range(c):
                            nc.vector.scalar_tensor_tensor(
                                out=Ut[:], in0=G[:, j * DROW:j * DROW + DH],
                                scalar=exm[:, j:j + 1], in1=Ut[:],
                                op0=mybir.AluOpType.mult,
                                op1=mybir.AluOpType.add)
                    dn = bp.tile([128, 1], f32, tag="dn")
                    nc.vector.tensor_reduce(
                        out=dn[:], in_=parts[:, :nch], axis=mybir.AxisListType.X,
                        op=mybir.AluOpType.add)
                    nc.vector.tensor_scalar(
                        out=dn[:], in0=dn[:], scalar1=1e-30, scalar2=None,
                        op0=mybir.AluOpType.max)
                    rc = bp.tile([128, 1], f32, tag="rc")
                    nc.vector.reciprocal(out=rc[:], in_=dn[:])
                    us = bp.tile([128, DH], f32, tag="us")
                    nc.vector.tensor_scalar(
                        out=us[:], in0=Ut[:], scalar1=rc[:, 0:1], scalar2=None,
                        op0=mybir.AluOpType.mult)
                    uT_ps = pst.tile([128, 128], f32, tag="tr")
                    nc.tensor.transpose(uT_ps[:], us[:], ident[:])
                    nc.vector.scalar_tensor_tensor(
                        out=hT[:, ts(p, 128)], in0=uT_ps[:],
                        scalar=bvs[:, 1 + l:2 + l], in1=hT[:, ts(p, 128)],
                        op0=mybir.AluOpType.add, op1=mybir.AluOpType.add)
                    col += Dbp

            # ---- mlp2 + sigmoid + masked partial sum
            for t in range(BPC):
                y_ps = ps.tile([128, DH + 2], f32, tag="mm")
                nc.tensor.matmul(out=y_ps[0:2, 0:128], lhsT=w2s[:],
                                 rhs=hT[:, ts(t, 128)], start=True, stop=True)
                y_sb = sm.tile([2, 128], f32, tag="y_sb")
                nc.scalar.activation(
                    out=y_sb[:], in_=y_ps[0:2, 0:128],
                    func=mybir.ActivationFunctionType.Sigmoid,
                    bias=b2s[:, 0:1], scale=1.0,
                    accum_out=fparts[:, t:t + 1])
            acc = pp.tile([2, 1], f32, name="acc")
            nc.vector.tensor_reduce(
                out=acc[:], in_=fparts[:, :BPC], axis=mybir.AxisListType.X,
                op=mybir.AluOpType.add)
            ob_in = dp.tile([2, 1], f32, name="ob_in")
            ob_out = dp.tile([2, 1], f32, name="ob_out")
            nc.sync.dma_start(ob_in[:, :], acc[:])
            nc.gpsimd.collective_compute(
                "AllReduce", mybir.AluOpType.add,
                replica_groups=[list(range(CORES))],
                ins=[ob_in[:, :].opt()],
                outs=[ob_out[:, :].opt()],
            )
            nc.sync.dma_start(o[:, :], ob_out[:, :])
    nc.finalize()
    return nc


# ------------------------------------------------------------------- runner
def _make_runner(nc):
    import jax
    from jax.experimental.shard_map import shard_map
    from jax.sharding import Mesh, NamedSharding, PartitionSpec
    import concourse.mybir as mybir
    from concourse import bass2jax

    bass2jax.install_neuronx_cc_hook()
    pname = nc.partition_id_tensor.name if nc.partition_id_tensor else None
    in_names, out_names, out_avals, out_shapes = [], [], [], []
    for alloc in nc.m.functions[0].allocations:
        if not isinstance(alloc, mybir.MemoryLocationSet):
            continue
        name = alloc.memorylocations[0].name
        if alloc.kind == "ExternalInput":
            if name != pname:
                in_names.append(name)
        elif alloc.kind == "ExternalOutput":
            out_names.append(name)
            shape = tuple(alloc.tensor_shape)
            dtype = mybir.dt.np(alloc.dtype)
            out_avals.append(jax.core.ShapedArray(shape, dtype))
            out_shapes.append((shape, dtype))
    all_in = in_names + out_names + ([pname] if pname else [])

    def _body(*args):
        operands = list(args)
        if pname:
            operands.append(bass2jax.partition_id_tensor())
        outs = bass2jax._bass_exec_p.bind(
            *operands, out_avals=tuple(out_avals), in_names=tuple(all_in),
            out_names=tuple(out_names), lowering_input_output_aliases=(),
            sim_require_finite=True, sim_require_nnan=True, nc=nc)
        return tuple(outs)

    devices = jax.devices()[:CORES]
    mesh = Mesh(np.asarray(devices), ("core",))
    specs_in = (PartitionSpec("core"),) * (len(in_names) + len(out_names))
    specs_out = (PartitionSpec("core"),) * len(out_names)
    sharded = jax.jit(
        shard_map(_body, mesh=mesh, in_specs=specs_in, out_specs=specs_out,
                  check_rep=False),
        keep_unused=True)
    sharding = NamedSharding(mesh, PartitionSpec("core"))

    dev_cache = {}   # name -> (host_concat_array, device_array)
    # o is fully DMA-written by the program, so the pre-zeroed "output input"
    # can be a single persistent device buffer reused every call (no H2D).
    dev_zeros = jax.device_put(
        [np.zeros((CORES * s[0], *s[1:]), d) for (s, d) in out_shapes],
        [sharding] * len(out_shapes))

    aot = [None]   # AOT-compiled executable, built on first launch

    def run(concat_in: dict | None):
        global launch_ns
        t0 = time.perf_counter()
        if concat_in is not None:
            stale = []
            for nm in in_names:
                ent = dev_cache.get(nm)
                if ent is None or not (
                        ent[0] is concat_in[nm]
                        or np.array_equal(ent[0], concat_in[nm])):
                    stale.append(nm)
            if stale:
                put = jax.device_put([concat_in[nm] for nm in stale],
                                     [sharding] * len(stale))
                for nm, d in zip(stale, put):
                    dev_cache[nm] = (concat_in[nm], d)
        args = [dev_cache[nm][1] for nm in in_names] + list(dev_zeros)
        if aot[0] is None:
            try:
                aot[0] = sharded.lower(*args).compile()
            except Exception:
                aot[0] = sharded   # fall back to the pjit path
        out_arrs = aot[0](*args)
        # o is AllReduce'd on device: every core holds the total; read shard 0
        res = np.asarray(out_arrs[0].addressable_shards[0].data)
        launch_ns += int((time.perf_counter() - t0) * 1e9)
        return res.reshape(2)

    return run


# ------------------------------------------------------------------- kernel
def kernel(x, edge_index, batch, W1, b1, Wg, att_src, att_dst, bg, W2, b2):
    x = np.ascontiguousarray(np.asarray(x, np.float32))
    W1 = np.asarray(W1, np.float32); b1 = np.asarray(b1, np.float32)
    Wg = np.asarray(Wg, np.float32)
    att_src = np.asarray(att_src, np.float32)
    att_dst = np.asarray(att_dst, np.float32)
    bg = np.asarray(bg, np.float32)
    W2 = np.asarray(W2, np.float32); b2 = np.asarray(b2, np.float32)
    ei = np.ascontiguousarray(np.asarray(edge_index))

    # ---- graph preprocessing (cached on edge_index content)
    g = _cache.get("graph")
    if g is None or not (g[0] is ei or np.array_equal(g[0], ei)):
        g = (ei, _preprocess(ei))
        _cache["graph"] = g
    pre = g[1]

    key = pre["rounds"]
    prog = _cache.get("prog")
    if prog is None or prog[0] != key:
        nc = _build_program(pre["rounds"])
        prog = (key, nc, _make_runner(nc))
        _cache["prog"] = prog
    _, nc, run = prog

    # ---- per-call input assembly, memoized on raw input content
    def _same(a, b):
        return a is b or (a.shape == b.shape and np.array_equal(a, b))

    raw = _cache.get("raw")
    wts = (W1, b1, Wg, att_src, att_dst, bg, W2, b2)
    hit = (raw is not None and _same(raw[0], x) and raw[1] is pre
           and all(_same(a, b) for a, b in zip(raw[2], wts)))
    if hit:
        total = run(None)
    else:
        old = pre["old_of_row"]
        xfull = np.zeros((NPAD, DH), np.float32)
        valid = old >= 0
        xfull[valid] = x[old[valid]]

        va_s = np.einsum("lij,lj->li", Wg, att_src)      # [L,DH]
        va_d = np.einsum("lij,lj->li", Wg, att_dst)
        wcat = np.concatenate([Wg, va_s[:, :, None], va_d[:, :, None]],
                              axis=2).reshape(L * DH, DH + 2).astype(np.float32)
        bvec = np.concatenate([b1[:, None], bg.T], axis=1).astype(np.float32)
        b2v = b2.reshape(2, 1).astype(np.float32)

        rep = lambda a: np.concatenate([a] * CORES, axis=0)
        concat_in = {
            "xin": xfull,
            "idx": np.ascontiguousarray(
                pre["idx_all"].reshape(CORES * 128, pre["R"])),
            "w1": rep(W1), "wcat": rep(wcat), "w2": rep(W2),
            "bvec": rep(bvec), "b2v": rep(b2v),
        }
        total = run(concat_in)
        _cache["raw"] = (x, pre, tuple(np.copy(w) for w in wts))

    # exact dummy-node correction: h_dummy = b1 + sum(bg), no edges ever
    h_dummy = b1 + bg.sum(axis=0)
    y_dummy = 1.0 / (1.0 + np.exp(-(h_dummy @ W2 + b2)))
    return (total - (NPAD - N) * y_dummy).astype(np.float32)
